# revision 1
# baseline (speedup 1.0000x reference)
"""BlockWiseHistogramEncoder Trainium2 kernel.

Input  x: [16, 1, 512, 512] int32, values in [0, 64).
Output:   [16, 1024, 65] float32. Image is split into 32x32 non-overlapping
16x16 blocks (row-major block order); out[b, l, 1+v] = count(v in block l)/256,
out[b, l, 0] = 0.

Sharding: pure data parallel over batch — 2 batches per core on 8 cores.

Per-core algorithm: SBUF tiles hold 128 blocks (partition = block) with the
block's 256 elements along the free dim (gathered by a strided DMA); GPSIMD
converts them to bf16. Counting is split across two engines in parallel:
  - VectorE: for low classes, tensor_scalar(is_equal, reduce-add accum_out)
    gives per-block counts at 4x 16-bit rate (one instruction per class).
  - ScalarE: for high classes, ACTIVATE(Sign, bias=-(c-0.5), accum_out) gives
    S'_c = #(v>=c) - #(v<c); adjacent differences (S'_c - S'_{c+1})/2 recover
    counts. S'_64 = -256 is a constant column.
GPSIMD does the S' differencing and the final 1/256 (1/512 for the
sign-derived columns) scaling.
"""
import sys

if "/opt/trn_rl_repo" not in sys.path:
    sys.path.insert(0, "/opt/trn_rl_repo")

import numpy as np

N_CORES = 8
B_PER_CORE = 2
H = W = 512
NC_CLS = 64
BLK = 16
HB = H // BLK          # 32 blocks per side
L = HB * HB            # 1024 blocks
E = BLK * BLK          # 256 elems per block
TILES = L // 128       # 8 tiles of 128 blocks per batch

N_ACT = 24             # classes 64-N_ACT..63 counted on ScalarE
N_DVE = NC_CLS - N_ACT # classes 0..N_DVE-1 counted on VectorE

_nc_cache = None
_run_cache = None


def _build():
    import concourse.bacc as bacc
    import concourse.mybir as mybir
    import concourse.tile as tile

    nc = bacc.Bacc("TRN2", target_bir_lowering=False, debug=False)
    x = nc.dram_tensor("x_in", [B_PER_CORE, H, W], mybir.dt.int32,
                       kind="ExternalInput")
    y = nc.dram_tensor("y_out", [B_PER_CORE, L, NC_CLS + 1], mybir.dt.float32,
                       kind="ExternalOutput")

    with tile.TileContext(nc) as tc:
        with tc.tile_pool(name="cst", bufs=1) as c_pool, \
             tc.tile_pool(name="io", bufs=6) as io_pool, \
             tc.tile_pool(name="wk", bufs=3) as w_pool, \
             tc.tile_pool(name="hs", bufs=4) as h_pool:
            # per-class ACT biases: -(c - 0.5) for c in [N_DVE, 63]
            bias = c_pool.tile([128, N_ACT], mybir.dt.float32)
            for j in range(N_ACT):
                c = N_DVE + j
                nc.vector.memset(bias[:, j:j + 1], -(c - 0.5))
            NT = B_PER_CORE * TILES
            xbs = [x.ap()[b].rearrange("(bh r) (bw c) -> bh bw r c",
                                       r=BLK, c=BLK)
                   for b in range(B_PER_CORE)]

            state = {}

            def load_stage(g):
                b, t = divmod(g, TILES)
                t_in = io_pool.tile([128, E], mybir.dt.int32)
                for i in range(4):
                    dst = t_in[32 * i:32 * (i + 1), :].rearrange(
                        "bw (r c) -> bw r c", c=BLK)
                    nc.sync.dma_start(dst, xbs[b][4 * t + i])
                t_bf = w_pool.tile([128, E], mybir.dt.bfloat16)
                nc.gpsimd.tensor_copy(t_bf[:], t_in[:])
                state[g] = t_bf

            def count_stage(g):
                b, t = divmod(g, TILES)
                t_bf = state.pop(g)
                t_h = h_pool.tile([128, NC_CLS + 1], mybir.dt.float32)
                nc.gpsimd.memset(t_h[:, 0:1], 0.0)
                # S' columns: j=0..N_ACT-1 from ACT, col N_ACT = -256
                t_s = h_pool.tile([128, N_ACT + 1], mybir.dt.float32, tag="s")
                nc.gpsimd.memset(t_s[:, N_ACT:N_ACT + 1], -256.0)
                t_tr = w_pool.tile([128, E], mybir.dt.bfloat16, tag="tr")
                t_ta = w_pool.tile([128, E], mybir.dt.bfloat16, tag="ta")
                for c in range(N_DVE):
                    nc.vector.tensor_scalar(
                        t_tr[:], t_bf[:], float(c), 0.0,
                        mybir.AluOpType.is_equal, mybir.AluOpType.add,
                        accum_out=t_h[:, c + 1:c + 2])
                for j in range(N_ACT):
                    nc.scalar.activation(
                        t_ta[:], t_bf[:],
                        mybir.ActivationFunctionType.Sign,
                        bias=bias[:, j:j + 1], scale=1.0,
                        accum_out=t_s[:, j:j + 1])
                # counts for ACT classes: (S'_c - S'_{c+1}) -> cols
                nc.gpsimd.tensor_sub(
                    t_h[:, N_DVE + 1:NC_CLS + 1],
                    t_s[:, 0:N_ACT], t_s[:, 1:N_ACT + 1])
                # final scaling: DVE cols /256, ACT cols /512
                nc.gpsimd.tensor_scalar_mul(
                    t_h[:, 1:N_DVE + 1], t_h[:, 1:N_DVE + 1], 1.0 / E)
                nc.gpsimd.tensor_scalar_mul(
                    t_h[:, N_DVE + 1:NC_CLS + 1],
                    t_h[:, N_DVE + 1:NC_CLS + 1], 1.0 / (2 * E))
                nc.sync.dma_start(y.ap()[b, 128 * t:128 * (t + 1)], t_h[:])

            # software pipeline: emit tile g+1's load/convert before tile g's
            # count/epilogue so the in-order gpsimd queue never blocks the
            # next tile's convert behind this tile's diff.
            load_stage(0)
            for g in range(NT):
                if g + 1 < NT:
                    load_stage(g + 1)
                count_stage(g)
    nc.compile()
    return nc


def _get_nc():
    global _nc_cache
    if _nc_cache is None:
        _nc_cache = _build()
    return _nc_cache


def _get_runner():
    """Build the sharded jitted executable once (run_bass_via_pjrt retraces
    per call otherwise)."""
    global _run_cache
    if _run_cache is not None:
        return _run_cache

    import jax
    from jax.sharding import Mesh, PartitionSpec
    from jax.experimental.shard_map import shard_map
    import concourse.mybir as mybir
    from concourse.bass2jax import (
        _bass_exec_p, install_neuronx_cc_hook, partition_id_tensor)

    nc = _get_nc()
    install_neuronx_cc_hook()

    partition_name = (nc.partition_id_tensor.name
                      if nc.partition_id_tensor else None)
    in_names, out_names, out_avals = [], [], []
    for alloc in nc.m.functions[0].allocations:
        if not isinstance(alloc, mybir.MemoryLocationSet):
            continue
        name = alloc.memorylocations[0].name
        if alloc.kind == "ExternalInput":
            if name != partition_name:
                in_names.append(name)
        elif alloc.kind == "ExternalOutput":
            out_names.append(name)
            out_avals.append(jax.core.ShapedArray(
                tuple(alloc.tensor_shape), mybir.dt.np(alloc.dtype)))
    n_params = len(in_names)
    n_outs = len(out_avals)
    all_in_names = list(in_names) + list(out_names)
    if partition_name is not None:
        all_in_names.append(partition_name)

    def _body(*args):
        operands = list(args)
        if partition_name is not None:
            operands.append(partition_id_tensor())
        outs = _bass_exec_p.bind(
            *operands,
            out_avals=tuple(out_avals),
            in_names=tuple(all_in_names),
            out_names=tuple(out_names),
            lowering_input_output_aliases=(),
            sim_require_finite=True,
            sim_require_nnan=True,
            nc=nc,
        )
        return tuple(outs)

    devices = jax.devices()[:N_CORES]
    mesh = Mesh(np.asarray(devices), ("core",))
    in_specs = (PartitionSpec("core"),) * (n_params + n_outs)
    out_specs = (PartitionSpec("core"),) * n_outs
    donate = tuple(range(n_params, n_params + n_outs))
    sharded = jax.jit(
        shard_map(_body, mesh=mesh, in_specs=in_specs, out_specs=out_specs,
                  check_rep=False),
        donate_argnums=donate, keep_unused=True)

    zero_shapes = [(N_CORES * a.shape[0], *a.shape[1:]) for a in out_avals]
    zero_dtypes = [a.dtype for a in out_avals]

    def run(concat_inputs):
        zeros = [np.zeros(s, d) for s, d in zip(zero_shapes, zero_dtypes)]
        out_arrs = sharded(*concat_inputs, *zeros)
        return {name: np.asarray(out_arrs[i]) for i, name in
                enumerate(out_names)}

    _run_cache = run
    return run


def kernel(x: np.ndarray) -> np.ndarray:
    assert x.shape == (16, 1, H, W) and x.dtype == np.int32, (x.shape, x.dtype)
    run = _get_runner()
    xs = np.ascontiguousarray(x[:, 0])          # [16, 512, 512] = concat of
    out = run([xs])["y_out"]                    # 8 cores' [2, 512, 512]
    return out.reshape(16, L, NC_CLS + 1).astype(np.float32, copy=False)



# revision 2
# speedup vs baseline: 3.6483x; 3.6483x over previous
"""BlockWiseHistogramEncoder Trainium2 kernel, v2 (digit-factorized PE design).

Input  x: [16, 1, 512, 512] int32, values in [0, 64).
Output:   [16, 1024, 65] float32. out[b, l, 1+v] = count(v in block l)/256,
out[b, l, 0] = 0. Blocks are 16x16, row-major (32x32 grid).

Sharding: pure data parallel over batch - 2 batches per core on 8 cores.

Per-core algorithm (2 batches, 2048 blocks of 256 elems):
  1. DMA-load block tiles [128 blocks, 256 elems] int32 (16 tiles).
  2. GPSIMD converts int32 -> int16.
  3. PE transposes each tile to element-major layout XT[eh][e128, l]
     (two 128x128 transposes per tile, int16 via identity matmul), DVE
     copies PSUM->SBUF.
  4. DVE builds digit masks at 4x: value v = 8*hi + lo;
     U_h[e,l] = ((v>>3)==h), V_d[e,l] = ((v&7)==d) as bf16 (16 masks).
  5. PE computes per-block joint counts: for each group of G=4 blocks,
     out[(h,l),(l',d)] = sum_e U[e,(h,l)] * V[e,(l',d)] via 2 PSUM-
     accumulated matmuls (e-halves). Groups are packed 4-per-PSUM-strip
     with tile_position column tiling, 16 supergroups per PSUM bank.
  6. ScalarE copies each PSUM bank to SBUF fp32 with scale 1/256.
  7. DVE extracts the l'==l diagonal with 4 partition-strided copies
     (partitions p = 32q+4h+l, so l = p%4; copy f-slice l'=lam from
     partitions p%4==lam).
  8. DMA scatters counts to y[b, l, 1+8h+d] (y pre-zeroed once).
"""
import sys

if "/opt/trn_rl_repo" not in sys.path:
    sys.path.insert(0, "/opt/trn_rl_repo")

import numpy as np

N_CORES = 8
B_PER_CORE = 2
H = W = 512
NC_CLS = 64
BLK = 16
HB = H // BLK          # 32 blocks per side
L = HB * HB            # 1024 blocks per batch
E = BLK * BLK          # 256 elems per block
TILES = L // 128       # 8 tiles of 128 blocks per batch

G = 4                  # blocks per matmul group (stationary = 32 cols)
NQ = 4                 # col-tiled groups per 32x128 PSUM strip set
SG = 16                # supergroups per PSUM bank (16*4*G = 256 blocks)
BLOCKS_PER_BANK = SG * NQ * G  # 256
BANKS_PER_BATCH = L // BLOCKS_PER_BANK  # 4

# within-32 partition permutation (q,h,l) -> (q,l,h):
# out partition i = 8l+h reads in partition 4h+l
SHUF = [4 * (i % 8) + i // 8 for i in range(32)]

_nc_cache = None
_run_cache = None


def _build():
    import concourse.bacc as bacc
    import concourse.mybir as mybir
    import concourse.tile as tile

    fp32 = mybir.dt.float32
    bf16 = mybir.dt.bfloat16
    i32 = mybir.dt.int32
    i16 = mybir.dt.int16

    nc = bacc.Bacc("TRN2", target_bir_lowering=False, debug=False)
    x = nc.dram_tensor("x_in", [B_PER_CORE, H, W], i32, kind="ExternalInput")
    # 64-wide (no zero class-0 col): keeps the out-scatter AP affine/3-dim.
    # Host prepends the constant zero column.
    y = nc.dram_tensor("y_out", [B_PER_CORE, L, NC_CLS], fp32,
                       kind="ExternalOutput")

    with tile.TileContext(nc) as tc:
        with tc.tile_pool(name="cst", bufs=1) as c_pool, \
             tc.tile_pool(name="io", bufs=4) as io_pool, \
             tc.tile_pool(name="cv", bufs=4) as cv_pool, \
             tc.tile_pool(name="xt", bufs=1) as xt_pool, \
             tc.tile_pool(name="mk", bufs=1) as mk_pool, \
             tc.tile_pool(name="ex", bufs=3) as ex_pool, \
             tc.tile_pool(name="ptr", bufs=4, space="PSUM") as ptr_pool, \
             tc.tile_pool(name="pmm", bufs=3, space="PSUM") as pmm_pool:

            # ---- constants ----
            # bf16 identity for PE transposes: iota(f - p) == 0
            ident = c_pool.tile([128, 128], bf16)
            iot = c_pool.tile([128, 128], i16, tag="iota")
            nc.gpsimd.iota(iot[:], pattern=[[1, 128]], base=0,
                           channel_multiplier=-1)
            nc.vector.tensor_scalar(
                ident[:], iot[:], 0, None, mybir.AluOpType.is_equal)

            # per-partition selector masks for the diagonal extraction:
            # p = 32q + 4h + l  ->  l(p) = p%4; msk[lam][p,:] = [l(p)==lam]
            pidx = c_pool.tile([128, 128], i32, tag="pidx")
            nc.gpsimd.iota(pidx[:], pattern=[[0, 128]], base=0,
                           channel_multiplier=1)
            lsel = c_pool.tile([128, 128], i32, tag="lsel")
            nc.vector.tensor_scalar(
                lsel[:], pidx[:], 3, None, mybir.AluOpType.bitwise_and)
            msk = []
            for lam in range(G):
                m = c_pool.tile([128, 128], i32, tag=f"msk{lam}",
                                name=f"msk{lam}")
                nc.vector.tensor_scalar(
                    m[:], lsel[:], lam, None, mybir.AluOpType.is_equal)
                msk.append(m)

            # persistent per-batch tensors
            XT = [[xt_pool.tile([128, L], i16, tag=f"xt{b}{eh}",
                                name=f"xt{b}{eh}")
                   for eh in range(2)] for b in range(B_PER_CORE)]
            # U interleaved: [e, group, h, l] so per-group weight slices
            # (h-major, l-minor) merge to one contiguous free dim; per-h
            # builds write packed (g, l) slices at 4x.
            U = [[mk_pool.tile([128, L // G, 8, G], bf16, tag=f"u{b}{eh}",
                               name=f"u{b}{eh}")
                  for eh in range(2)] for b in range(B_PER_CORE)]
            # V interleaved: [e, group, d, l'] so per-group moving slices
            # (d-major, l'-minor) merge to one contiguous free dim, while
            # per-d mask builds still write packed (g, l') slices at 4x.
            V = [[mk_pool.tile([128, L // G, 8, G], bf16, tag=f"v{b}{eh}",
                               name=f"v{b}{eh}")
                  for eh in range(2)] for b in range(B_PER_CORE)]

            xbs = [x.ap()[b].rearrange("(bh r) (bw c) -> bh bw r c",
                                       r=BLK, c=BLK)
                   for b in range(B_PER_CORE)]

            def load_tile(b, t):
                # [128 blocks, 256 elems] int32, strided DMA (64B chunks)
                t_in = io_pool.tile([128, E], i32)
                for i in range(4):
                    dst = t_in[32 * i:32 * (i + 1), :].rearrange(
                        "bw (r c) -> bw r c", c=BLK)
                    nc.sync.dma_start(dst, xbs[b][4 * t + i])
                return t_in

            def convert_tile(t_in):
                # int32 -> bf16 (values < 64, exact)
                tb = cv_pool.tile([128, E], bf16)
                nc.gpsimd.tensor_copy(tb[:], t_in[:])
                return tb

            def transpose_tile(b, t, tb):
                # two 128x128 bf16 transposes -> PSUM, then DVE copies
                # PSUM->SBUF converting bf16 -> int16 (values < 64, exact)
                p_tr = ptr_pool.tile([128, 256], bf16)
                for eh in range(2):
                    nc.tensor.transpose(
                        p_tr[:, 128 * eh:128 * (eh + 1)],
                        tb[:, 128 * eh:128 * (eh + 1)], ident[:])
                for eh in range(2):
                    nc.vector.tensor_copy(
                        XT[b][eh][:, 128 * t:128 * (t + 1)],
                        p_tr[:, 128 * eh:128 * (eh + 1)])

            def build_masks(b, eh):
                xt = XT[b][eh]
                xh = cv_pool.tile([128, L], i16, tag="xh", name="xh")
                nc.vector.tensor_scalar(
                    xh[:], xt[:], 56, None, mybir.AluOpType.bitwise_and)
                xl = cv_pool.tile([128, L], i16, tag="xl", name="xl")
                nc.vector.tensor_scalar(
                    xl[:], xt[:], 7, None, mybir.AluOpType.bitwise_and)
                xhg = xh[:].rearrange("e (g l) -> e g l", l=G)
                xlg = xl[:].rearrange("e (g l) -> e g l", l=G)
                for h in range(8):
                    nc.vector.tensor_scalar(
                        U[b][eh][:, :, h, :], xhg, float(8 * h), None,
                        mybir.AluOpType.is_equal)
                for d in range(8):
                    nc.vector.tensor_scalar(
                        V[b][eh][:, :, d, :], xlg, float(d), None,
                        mybir.AluOpType.is_equal)

            def bank_groups(b, bank):
                # PE: fill one PSUM bank with SG supergroups x NQ col-strips
                pm = pmm_pool.tile([128, SG * 32], fp32)
                for sg in range(SG):
                    for q in range(NQ):
                        g0 = ((bank * SG + sg) * NQ + q) * G
                        out = pm[32 * q:32 * (q + 1), 32 * sg:32 * (sg + 1)]
                        for eh in range(2):
                            # lhsT: U [e128, (h8, l4)]; rhs: V [e128, (d8, l'4)]
                            # out[p=(h,l), f=(d,l')]; diag l'==l is useful
                            lhsT = U[b][eh][:, g0 // G]
                            rhs = V[b][eh][:, g0 // G]
                            nc.tensor.matmul(
                                out, lhsT, rhs,
                                start=(eh == 0), stop=(eh == 1),
                                tile_position=(0, 32 * q))
                return pm

            def extract_bank(b, bank, pm):
                # ScalarE: PSUM fp32 -> SBUF fp32 with 1/256 scaling
                c_sb = ex_pool.tile([128, SG * 32], fp32, tag="c")
                nc.scalar.activation(
                    c_sb[:], pm[:], mybir.ActivationFunctionType.Copy,
                    bias=0.0, scale=1.0 / E)
                # DVE: diagonal extraction -- for each lam, copy f-slice
                # l'=lam into partitions with l(p)==lam (predicated).
                st = ex_pool.tile([128, SG * 8], fp32, tag="st")
                c_v = c_sb[:].rearrange("p (sg d l) -> p sg d l", l=G, d=8)
                for lam in range(G):
                    nc.vector.copy_predicated(
                        st[:], msk[lam][:, :SG * 8], c_v[:, :, :, lam])
                # permute partitions (q,h,l) -> (q,l,h) within each 32-group
                # so the out-scatter AP is affine (32B per partition step)
                stp = ex_pool.tile([128, SG * 8], fp32, tag="stp")
                nc.vector.stream_shuffle(stp[:], st[:], SHUF)
                # DMA: stp[p=(q,l,h), (sg,d)] -> y64[b, l_glob, 8h+d]
                # l_glob = ((bank*SG+sg)*NQ+q)*G + l; affine: p-stride 32B,
                # sg-stride 4096B, d-stride 4B.
                dst = y.ap()[b].rearrange(
                    "(bank sg q l) (h d) -> bank (q l h) sg d",
                    sg=SG, q=NQ, l=G, h=8)
                src = stp[:].rearrange("p (sg d) -> p sg d", d=8)
                nc.scalar.dma_start(dst[bank], src)

            # ---------------- pipeline ----------------
            for b in range(B_PER_CORE):
                for t in range(TILES):
                    t_in = load_tile(b, t)
                    t16 = convert_tile(t_in)
                    transpose_tile(b, t, t16)
                for eh in range(2):
                    build_masks(b, eh)
                for bank in range(BANKS_PER_BATCH):
                    pm = bank_groups(b, bank)
                    extract_bank(b, bank, pm)

    nc.compile()
    return nc


def _get_nc():
    global _nc_cache
    if _nc_cache is None:
        _nc_cache = _build()
    return _nc_cache


def _get_runner():
    """Build the sharded jitted executable once."""
    global _run_cache
    if _run_cache is not None:
        return _run_cache

    import jax
    from jax.sharding import Mesh, PartitionSpec
    from jax.experimental.shard_map import shard_map
    import concourse.mybir as mybir
    from concourse.bass2jax import (
        _bass_exec_p, install_neuronx_cc_hook, partition_id_tensor)

    nc = _get_nc()
    install_neuronx_cc_hook()

    partition_name = (nc.partition_id_tensor.name
                      if nc.partition_id_tensor else None)
    in_names, out_names, out_avals = [], [], []
    for alloc in nc.m.functions[0].allocations:
        if not isinstance(alloc, mybir.MemoryLocationSet):
            continue
        name = alloc.memorylocations[0].name
        if alloc.kind == "ExternalInput":
            if name != partition_name:
                in_names.append(name)
        elif alloc.kind == "ExternalOutput":
            out_names.append(name)
            out_avals.append(jax.core.ShapedArray(
                tuple(alloc.tensor_shape), mybir.dt.np(alloc.dtype)))
    n_params = len(in_names)
    n_outs = len(out_avals)
    all_in_names = list(in_names) + list(out_names)
    if partition_name is not None:
        all_in_names.append(partition_name)

    def _body(*args):
        operands = list(args)
        if partition_name is not None:
            operands.append(partition_id_tensor())
        outs = _bass_exec_p.bind(
            *operands,
            out_avals=tuple(out_avals),
            in_names=tuple(all_in_names),
            out_names=tuple(out_names),
            lowering_input_output_aliases=(),
            sim_require_finite=True,
            sim_require_nnan=True,
            nc=nc,
        )
        return tuple(outs)

    devices = jax.devices()[:N_CORES]
    mesh = Mesh(np.asarray(devices), ("core",))
    in_specs = (PartitionSpec("core"),) * (n_params + n_outs)
    out_specs = (PartitionSpec("core"),) * n_outs
    donate = tuple(range(n_params, n_params + n_outs))
    sharded = jax.jit(
        shard_map(_body, mesh=mesh, in_specs=in_specs, out_specs=out_specs,
                  check_rep=False),
        donate_argnums=donate, keep_unused=True)

    zero_shapes = [(N_CORES * a.shape[0], *a.shape[1:]) for a in out_avals]
    zero_dtypes = [a.dtype for a in out_avals]

    def run(concat_inputs):
        zeros = [np.zeros(s, d) for s, d in zip(zero_shapes, zero_dtypes)]
        out_arrs = sharded(*concat_inputs, *zeros)
        return {name: np.asarray(out_arrs[i]) for i, name in
                enumerate(out_names)}

    _run_cache = run
    return run


def kernel(x: np.ndarray) -> np.ndarray:
    assert x.shape == (16, 1, H, W) and x.dtype == np.int32, (x.shape, x.dtype)
    run = _get_runner()
    xs = np.ascontiguousarray(x[:, 0])          # [16, 512, 512]
    out = run([xs])["y_out"].reshape(16, L, NC_CLS)
    full = np.zeros((16, L, NC_CLS + 1), np.float32)
    full[:, :, 1:] = out
    return full
